# revision 7
# baseline (speedup 1.0000x reference)
"""Trainium2 Bass kernel for nn_AdaBoostClassifier (8-core data-parallel).

Reference computation:
    logits = x @ W.T + b                      # [N, E]
    preds  = round(sigmoid(logits))           # {0,1} == 1[logit > 0]
    acc    = sum_e trunc(alpha_e) * preds_e   # integer-valued
    out    = sign(acc)

Math used here: with t = trunc(alphas), s_e = Sign(logit_e) in {-1,0,1}:
    acc = (sum_e t_e * s_e + sum_e t_e) / 2   (exact; s=0 measure-zero)
so out = Sign(dot(t, s) + T) with T = sum(t). Columns with t_e == 0
contribute nothing, so only those estimators are computed (selected on
host at runtime — valid for any input values).

Device pipeline per 512-sample block:
  3-pass mixed-precision matmul: xh=fp16(x) against Wh=fp16(W) and
  Wl=bf16(W-Wh), plus xl=fp8e4((x-xh)*2^12) against Ws=fp16(Wh*2^-12)
  (subnormal; scales cancel so all passes accumulate into one fp32 PSUM
  group; total precision ~2^-16, 1 output flip in 131072 vs fp32) ->
  ACT Sign(psum + b) -> PE matvec with t -> ACT Sign(acc + T) -> DMA out.
"""

import math

import numpy as np
import ml_dtypes

import concourse.bass as bass  # noqa: F401  (registers bass types)
import concourse.tile as tile
from concourse import bacc, mybir
from concourse.bass_utils import run_bass_kernel_spmd

BF16 = ml_dtypes.bfloat16
F8E4 = ml_dtypes.float8_e4m3
XL_SCALE = 2.0 ** 12
XL_INV_SCALE = 2.0 ** -12

N_CORES = 8
N_FULL = 131072
F_DIM = 512
NS = N_FULL // N_CORES          # samples per core
BLK = 512                       # samples per psum block (one PSUM bank)
SUPER = 1024                    # samples per DMA super-block
N_SUPER = NS // SUPER
SUBS = SUPER // BLK
KC = F_DIM // 128               # contraction chunks

_program_cache: dict[int, object] = {}


def _build(n_etiles: int):
    """Build the 8-core SPMD program for n_etiles 128-wide estimator tiles."""
    nc = bacc.Bacc("TRN2", target_bir_lowering=False, debug=False)

    d_xh = nc.dram_tensor("xh", [F_DIM, NS], mybir.dt.float16, kind="ExternalInput")
    d_xl = nc.dram_tensor("xl", [F_DIM, NS], mybir.dt.float8e4, kind="ExternalInput")
    d_wh = nc.dram_tensor(
        "wh", [n_etiles, F_DIM, 128], mybir.dt.float16, kind="ExternalInput"
    )
    d_wl = nc.dram_tensor(
        "wl", [n_etiles, F_DIM, 128], mybir.dt.bfloat16, kind="ExternalInput"
    )
    d_ws = nc.dram_tensor(
        "ws", [n_etiles, F_DIM, 128], mybir.dt.float16, kind="ExternalInput"
    )
    d_bv = nc.dram_tensor("bv", [n_etiles, 128, 1], mybir.dt.float32, kind="ExternalInput")
    d_tv = nc.dram_tensor("tv", [n_etiles, 128, 1], mybir.dt.bfloat16, kind="ExternalInput")
    d_tt = nc.dram_tensor("tt", [1, 1], mybir.dt.float32, kind="ExternalInput")
    d_out = nc.dram_tensor("out", [NS], mybir.dt.float32, kind="ExternalOutput")

    # f-major views of x planes: (k p) n -> p k n, so chunk k = features
    # [128k, 128k+128) on partitions.
    xh_v = d_xh.ap().rearrange("(k p) n -> p k n", p=128)
    xl_v = d_xl.ap().rearrange("(k p) n -> p k n", p=128)
    out_v = d_out.ap().rearrange("(s n) -> s n", n=SUPER)

    with tile.TileContext(nc) as tc:
        with (
            tc.tile_pool(name="singles", bufs=1) as singles,
            tc.tile_pool(name="xbuf", bufs=6) as xbuf,
            tc.tile_pool(name="sbuf", bufs=4) as spool,
            tc.tile_pool(name="obuf", bufs=3) as obuf,
            tc.tile_pool(name="pslog", bufs=3, space="PSUM") as pslog,
            tc.tile_pool(name="psacc", bufs=3, space="PSUM") as psacc,
        ):
            # --- weights / per-estimator constants: few batched DMAs on the
            # ACT HWDGE ring so the SP ring streams x from cycle 0 ---
            wh_t = singles.tile([128, n_etiles, KC, 128], mybir.dt.float16, tag="wh")
            nc.scalar.dma_start(
                out=wh_t, in_=d_wh.ap().rearrange("j (k p) e -> p j k e", p=128)
            )
            wl_t = singles.tile([128, n_etiles, KC, 128], mybir.dt.bfloat16, tag="wl")
            nc.scalar.dma_start(
                out=wl_t, in_=d_wl.ap().rearrange("j (k p) e -> p j k e", p=128)
            )
            ws_t = singles.tile([128, n_etiles, KC, 128], mybir.dt.float16, tag="ws")
            nc.scalar.dma_start(
                out=ws_t, in_=d_ws.ap().rearrange("j (k p) e -> p j k e", p=128)
            )
            bv_t = singles.tile([128, n_etiles], mybir.dt.float32, tag="bv")
            nc.scalar.dma_start(
                out=bv_t, in_=d_bv.ap().rearrange("j p one -> p (j one)")
            )
            tv_t = singles.tile([128, n_etiles], mybir.dt.bfloat16, tag="tv")
            nc.scalar.dma_start(
                out=tv_t, in_=d_tv.ap().rearrange("j p one -> p (j one)")
            )
            tt_t = singles.tile([1, 1], mybir.dt.float32, tag="tt")
            nc.scalar.dma_start(out=tt_t, in_=d_tt.ap())

            # --- main loop ---
            # Stage-2 (t-matvec + output Sign + out-DMA) for sub-block i is
            # emitted while sub-block i+1's stage-1 matmuls run, so the PE
            # never idles waiting for ACT's Sign output.
            pending = []  # (s_tiles per j, out_sb, ns, sb) awaiting stage 2
            out_tiles = {}

            def flush_stage2():
                s_tiles, o_sb, ns_, _sb = pending.pop(0)
                acc = psacc.tile([1, BLK], mybir.dt.float32, tag="acc",
                                 name=f"acc{_sb}_{ns_.start}")
                for j in range(n_etiles):
                    nc.tensor.matmul(
                        acc, tv_t[:, j:j + 1], s_tiles[j],
                        start=(j == 0), stop=(j == n_etiles - 1),
                    )
                nc.scalar.activation(
                    out=o_sb[0:1, ns_], in_=acc,
                    func=mybir.ActivationFunctionType.Sign,
                    bias=tt_t,
                )

            for sb in range(N_SUPER):
                n0 = sb * SUPER
                xh_sb = xbuf.tile([128, KC, SUPER], mybir.dt.float16, tag="xh",
                                  name=f"xh{sb}")
                xl_sb = xbuf.tile([128, KC, SUPER], mybir.dt.float8e4, tag="xl",
                                  name=f"xl{sb}")
                if sb == 0:
                    for s in range(SUBS):
                        lo, hi = s * BLK, (s + 1) * BLK
                        nc.sync.dma_start(
                            out=xh_sb[:, :, lo:hi],
                            in_=xh_v[:, :, n0 + lo:n0 + hi])
                        nc.sync.dma_start(
                            out=xl_sb[:, :, lo:hi],
                            in_=xl_v[:, :, n0 + lo:n0 + hi])
                else:
                    nc.sync.dma_start(out=xh_sb, in_=xh_v[:, :, n0:n0 + SUPER])
                    nc.sync.dma_start(out=xl_sb, in_=xl_v[:, :, n0:n0 + SUPER])

                out_sb = obuf.tile([1, SUPER], mybir.dt.float32, tag="osb",
                                   name=f"osb{sb}")
                out_tiles[sb] = out_sb

                for s in range(SUBS):
                    ns = slice(s * BLK, (s + 1) * BLK)
                    s_tiles = []
                    for j in range(n_etiles):
                        logits = pslog.tile([128, BLK], mybir.dt.float32, tag="lg",
                                            name=f"lg{sb}_{s}_{j}")
                        # xh-dependent passes first so the PE can start as
                        # soon as xh lands, while xl is still streaming in.
                        passes = [(wh_t, xh_sb), (wl_t, xh_sb), (ws_t, xl_sb)]
                        mm = 0
                        for w_t, x_sb in passes:
                            for k in range(KC):
                                nc.tensor.matmul(
                                    logits, w_t[:, j, k, :], x_sb[:, k, ns],
                                    start=(mm == 0), stop=(mm == 3 * KC - 1),
                                )
                                mm += 1
                        s_t = spool.tile([128, BLK], mybir.dt.bfloat16, tag="sg",
                                         name=f"sg{sb}_{s}_{j}")
                        nc.scalar.activation(
                            out=s_t, in_=logits,
                            func=mybir.ActivationFunctionType.Sign,
                            bias=bv_t[:, j:j + 1],
                        )
                        s_tiles.append(s_t)
                    pending.append((s_tiles, out_sb, ns, sb))
                    if len(pending) > 1:
                        flush_stage2()
                # previous super-block's outputs are all signed by now
                if sb > 0:
                    nc.scalar.dma_start(
                        out=out_v[sb - 1:sb, :], in_=out_tiles[sb - 1]
                    )
            while pending:
                flush_stage2()
            nc.scalar.dma_start(
                out=out_v[N_SUPER - 1:N_SUPER, :], in_=out_tiles[N_SUPER - 1]
            )

    nc.compile()
    return nc


def _prep_inputs(x, W, b, alphas):
    """Host-side prep: estimator selection, transposes, hi/lo splits."""
    t_full = np.trunc(alphas.astype(np.float32)).astype(np.float32)
    T = float(t_full.sum())
    nz = np.flatnonzero(t_full)
    n_etiles = max(1, math.ceil(len(nz) / 128))
    e_pad = n_etiles * 128

    W_sel = np.zeros((e_pad, F_DIM), np.float32)
    b_sel = np.zeros((e_pad,), np.float32)
    t_sel = np.zeros((e_pad,), np.float32)
    if len(nz):
        W_sel[: len(nz)] = W[nz]
        b_sel[: len(nz)] = b[nz]
        t_sel[: len(nz)] = t_full[nz]

    # [n_etiles, F, 128] stationary layout (partition = feature)
    w_fe = W_sel.T.reshape(F_DIM, n_etiles, 128).transpose(1, 0, 2)
    wh = w_fe.astype(np.float16)
    wl = (w_fe - wh.astype(np.float32)).astype(BF16)
    # scaled copy of wh for the fp8 xl pass: products (xl*2^12)*(wh*2^-12)
    # accumulate unscaled into the same PSUM group. Subnormal fp16 is fine
    # (PE handles it; pass-3's W only needs ~8 bits since xl ~ 2^-12 |x|).
    ws = (wh.astype(np.float32) * XL_INV_SCALE).astype(np.float16)

    xT = np.ascontiguousarray(x.T.astype(np.float32))  # [F, N]
    xh = xT.astype(np.float16)
    xl = ((xT - xh.astype(np.float32)) * XL_SCALE).astype(F8E4)

    bv = np.ascontiguousarray(b_sel.reshape(n_etiles, 128, 1))
    tv = np.ascontiguousarray(t_sel.reshape(n_etiles, 128, 1)).astype(BF16)
    tt = np.array([[T]], np.float32)

    in_maps = []
    for c in range(N_CORES):
        sl = slice(c * NS, (c + 1) * NS)
        in_maps.append({
            "xh": np.ascontiguousarray(xh[:, sl]),
            "xl": np.ascontiguousarray(xl[:, sl]),
            "wh": wh, "wl": wl, "ws": ws, "bv": bv, "tv": tv, "tt": tt,
        })
    return n_etiles, in_maps


def kernel(x, W, b, alphas, _trace=False, _trace_kwargs=None):
    n_etiles, in_maps = _prep_inputs(
        np.asarray(x), np.asarray(W), np.asarray(b), np.asarray(alphas)
    )
    nc = _program_cache.get(n_etiles)
    if nc is None:
        nc = _build(n_etiles)
        _program_cache[n_etiles] = nc

    kwargs = {}
    if _trace:
        kwargs["trace"] = True
        kwargs.update(_trace_kwargs or {})
    res = run_bass_kernel_spmd(nc, in_maps, core_ids=list(range(N_CORES)), **kwargs)
    out = np.concatenate([res.results[c]["out"] for c in range(N_CORES)])
    if _trace:
        kernel.last_results = res
    return out.astype(np.float32)


# revision 8
# speedup vs baseline: 1.1134x; 1.1134x over previous
"""Trainium2 Bass kernel for nn_AdaBoostClassifier (8-core data-parallel).

Reference computation:
    logits = x @ W.T + b                      # [N, E]
    preds  = round(sigmoid(logits))           # {0,1} == 1[logit > 0]
    acc    = sum_e trunc(alpha_e) * preds_e   # integer-valued
    out    = sign(acc)

Math used here: with t = trunc(alphas), s_e = Sign(logit_e) in {-1,0,1}:
    acc = (sum_e t_e * s_e + sum_e t_e) / 2   (exact; s=0 measure-zero)
so out = Sign(dot(t, s) + T) with T = sum(t). Columns with t_e == 0
contribute nothing, so only those estimators are computed (selected on
host at runtime — valid for any input values).

Device pipeline per 512-sample block:
  3-pass mixed-precision matmul: xh=fp16(x) against Wh=fp16(W) and
  Wl=bf16(W-Wh), plus xl=fp8e4((x-xh)*2^12) against Ws=fp16(Wh*2^-12)
  (subnormal; scales cancel so all passes accumulate into one fp32 PSUM
  group; total precision ~2^-16, 1 output flip in 131072 vs fp32) ->
  ACT Sign(psum + b) -> PE matvec with t -> ACT Sign(acc + T) -> DMA out.
"""

import math

import numpy as np
import ml_dtypes

import concourse.bass as bass  # noqa: F401  (registers bass types)
import concourse.tile as tile
from concourse import bacc, mybir
from concourse.bass_utils import run_bass_kernel_spmd

import os

BF16 = ml_dtypes.bfloat16
F8E4 = ml_dtypes.float8_e4m3
XL_FP8 = os.environ.get("KERNEL_XL_FP8", "1") == "1"
XL_SCALE = 2.0 ** 12 if XL_FP8 else 1.0
XL_INV_SCALE = 1.0 / XL_SCALE
XL_DT = mybir.dt.float8e4 if XL_FP8 else mybir.dt.bfloat16
XL_NP = F8E4 if XL_FP8 else BF16

N_CORES = 8
N_FULL = 131072
F_DIM = 512
NS = N_FULL // N_CORES          # samples per core
BLK = 512                       # samples per psum block (one PSUM bank)
SUPER = 1024                    # samples per DMA super-block
N_SUPER = NS // SUPER
SUBS = SUPER // BLK
KC = F_DIM // 128               # contraction chunks

_program_cache: dict[int, object] = {}


def _build(n_etiles: int):
    """Build the 8-core SPMD program for n_etiles 128-wide estimator tiles."""
    nc = bacc.Bacc("TRN2", target_bir_lowering=False, debug=False)

    d_xh = nc.dram_tensor("xh", [F_DIM, NS], mybir.dt.float16, kind="ExternalInput")
    d_xl = nc.dram_tensor("xl", [F_DIM, NS], XL_DT, kind="ExternalInput")
    d_wh = nc.dram_tensor(
        "wh", [n_etiles, F_DIM, 128], mybir.dt.float16, kind="ExternalInput"
    )
    d_wl = nc.dram_tensor(
        "wl", [n_etiles, F_DIM, 128], mybir.dt.bfloat16, kind="ExternalInput"
    )
    d_ws = nc.dram_tensor(
        "ws", [n_etiles, F_DIM, 128], mybir.dt.float16, kind="ExternalInput"
    )
    d_bv = nc.dram_tensor("bv", [n_etiles, 128, 1], mybir.dt.float32, kind="ExternalInput")
    d_tv = nc.dram_tensor("tv", [n_etiles, 128, 1], mybir.dt.bfloat16, kind="ExternalInput")
    d_tt = nc.dram_tensor("tt", [1, 1], mybir.dt.float32, kind="ExternalInput")
    d_out = nc.dram_tensor("out", [NS], mybir.dt.float32, kind="ExternalOutput")

    # f-major views of x planes: (k p) n -> p k n, so chunk k = features
    # [128k, 128k+128) on partitions.
    xh_v = d_xh.ap().rearrange("(k p) n -> p k n", p=128)
    xl_v = d_xl.ap().rearrange("(k p) n -> p k n", p=128)
    out_v = d_out.ap().rearrange("(s n) -> s n", n=SUPER)

    with tile.TileContext(nc) as tc:
        with (
            tc.tile_pool(name="singles", bufs=1) as singles,
            tc.tile_pool(name="xbuf", bufs=6) as xbuf,
            tc.tile_pool(name="sbuf", bufs=4) as spool,
            tc.tile_pool(name="obuf", bufs=3) as obuf,
            tc.tile_pool(name="pslog", bufs=3, space="PSUM") as pslog,
            tc.tile_pool(name="psacc", bufs=3, space="PSUM") as psacc,
        ):
            # --- weights / per-estimator constants: few batched DMAs on the
            # ACT HWDGE ring so the SP ring streams x from cycle 0 ---
            wh_t = singles.tile([128, n_etiles, KC, 128], mybir.dt.float16, tag="wh")
            nc.scalar.dma_start(
                out=wh_t, in_=d_wh.ap().rearrange("j (k p) e -> p j k e", p=128)
            )
            wl_t = singles.tile([128, n_etiles, KC, 128], mybir.dt.bfloat16, tag="wl")
            nc.scalar.dma_start(
                out=wl_t, in_=d_wl.ap().rearrange("j (k p) e -> p j k e", p=128)
            )
            ws_t = singles.tile([128, n_etiles, KC, 128], mybir.dt.float16, tag="ws")
            nc.scalar.dma_start(
                out=ws_t, in_=d_ws.ap().rearrange("j (k p) e -> p j k e", p=128)
            )
            bv_t = singles.tile([128, n_etiles], mybir.dt.float32, tag="bv")
            nc.scalar.dma_start(
                out=bv_t, in_=d_bv.ap().rearrange("j p one -> p (j one)")
            )
            tv_t = singles.tile([128, n_etiles], mybir.dt.bfloat16, tag="tv")
            nc.scalar.dma_start(
                out=tv_t, in_=d_tv.ap().rearrange("j p one -> p (j one)")
            )
            tt_t = singles.tile([1, 1], mybir.dt.float32, tag="tt")
            nc.scalar.dma_start(out=tt_t, in_=d_tt.ap())

            # --- main loop ---
            # Stage-2 (t-matvec + output Sign + out-DMA) for sub-block i is
            # emitted while sub-block i+1's stage-1 matmuls run, so the PE
            # never idles waiting for ACT's Sign output.
            pending = []  # (s_tiles per j, out_sb, ns, sb) awaiting stage 2
            out_tiles = {}

            def flush_stage2():
                s_tiles, o_sb, ns_, _sb = pending.pop(0)
                acc = psacc.tile([1, BLK], mybir.dt.float32, tag="acc",
                                 name=f"acc{_sb}_{ns_.start}")
                for j in range(n_etiles):
                    nc.tensor.matmul(
                        acc, tv_t[:, j:j + 1], s_tiles[j],
                        start=(j == 0), stop=(j == n_etiles - 1),
                    )
                nc.scalar.activation(
                    out=o_sb[0:1, ns_], in_=acc,
                    func=mybir.ActivationFunctionType.Sign,
                    bias=tt_t,
                )

            for sb in range(N_SUPER):
                n0 = sb * SUPER
                xh_sb = xbuf.tile([128, KC, SUPER], mybir.dt.float16, tag="xh",
                                  name=f"xh{sb}")
                xl_sb = xbuf.tile([128, KC, SUPER], XL_DT, tag="xl",
                                  name=f"xl{sb}")
                if sb == 0:
                    for s in range(SUBS):
                        lo, hi = s * BLK, (s + 1) * BLK
                        nc.sync.dma_start(
                            out=xh_sb[:, :, lo:hi],
                            in_=xh_v[:, :, n0 + lo:n0 + hi])
                        nc.sync.dma_start(
                            out=xl_sb[:, :, lo:hi],
                            in_=xl_v[:, :, n0 + lo:n0 + hi])
                else:
                    nc.sync.dma_start(out=xh_sb, in_=xh_v[:, :, n0:n0 + SUPER])
                    nc.sync.dma_start(out=xl_sb, in_=xl_v[:, :, n0:n0 + SUPER])

                out_sb = obuf.tile([1, SUPER], mybir.dt.float32, tag="osb",
                                   name=f"osb{sb}")
                out_tiles[sb] = out_sb

                for s in range(SUBS):
                    ns = slice(s * BLK, (s + 1) * BLK)
                    s_tiles = []
                    for j in range(n_etiles):
                        logits = pslog.tile([128, BLK], mybir.dt.float32, tag="lg",
                                            name=f"lg{sb}_{s}_{j}")
                        # xh-dependent passes first so the PE can start as
                        # soon as xh lands, while xl is still streaming in.
                        passes = [(wh_t, xh_sb), (wl_t, xh_sb), (ws_t, xl_sb)]
                        mm = 0
                        for w_t, x_sb in passes:
                            for k in range(KC):
                                nc.tensor.matmul(
                                    logits, w_t[:, j, k, :], x_sb[:, k, ns],
                                    start=(mm == 0), stop=(mm == 3 * KC - 1),
                                )
                                mm += 1
                        s_t = spool.tile([128, BLK], mybir.dt.bfloat16, tag="sg",
                                         name=f"sg{sb}_{s}_{j}")
                        nc.scalar.activation(
                            out=s_t, in_=logits,
                            func=mybir.ActivationFunctionType.Sign,
                            bias=bv_t[:, j:j + 1],
                        )
                        s_tiles.append(s_t)
                    pending.append((s_tiles, out_sb, ns, sb))
                    if len(pending) > 1:
                        flush_stage2()
                # previous super-block's outputs are all signed by now
                if sb > 0:
                    nc.scalar.dma_start(
                        out=out_v[sb - 1:sb, :], in_=out_tiles[sb - 1]
                    )
            while pending:
                flush_stage2()
            nc.scalar.dma_start(
                out=out_v[N_SUPER - 1:N_SUPER, :], in_=out_tiles[N_SUPER - 1]
            )

    nc.compile()
    return nc


def _prep_inputs(x, W, b, alphas):
    """Host-side prep: estimator selection, transposes, hi/lo splits."""
    t_full = np.trunc(alphas.astype(np.float32)).astype(np.float32)
    T = float(t_full.sum())
    nz = np.flatnonzero(t_full)
    n_etiles = max(1, math.ceil(len(nz) / 128))
    e_pad = n_etiles * 128

    W_sel = np.zeros((e_pad, F_DIM), np.float32)
    b_sel = np.zeros((e_pad,), np.float32)
    t_sel = np.zeros((e_pad,), np.float32)
    if len(nz):
        W_sel[: len(nz)] = W[nz]
        b_sel[: len(nz)] = b[nz]
        t_sel[: len(nz)] = t_full[nz]

    # [n_etiles, F, 128] stationary layout (partition = feature)
    w_fe = W_sel.T.reshape(F_DIM, n_etiles, 128).transpose(1, 0, 2)
    wh = w_fe.astype(np.float16)
    wl = (w_fe - wh.astype(np.float32)).astype(BF16)
    # scaled copy of wh for the fp8 xl pass: products (xl*2^12)*(wh*2^-12)
    # accumulate unscaled into the same PSUM group. Subnormal fp16 is fine
    # (PE handles it; pass-3's W only needs ~8 bits since xl ~ 2^-12 |x|).
    ws = (wh.astype(np.float32) * XL_INV_SCALE).astype(np.float16)

    xT = np.ascontiguousarray(x.T.astype(np.float32))  # [F, N]
    xh = xT.astype(np.float16)
    xl = ((xT - xh.astype(np.float32)) * XL_SCALE).astype(XL_NP)

    bv = np.ascontiguousarray(b_sel.reshape(n_etiles, 128, 1))
    tv = np.ascontiguousarray(t_sel.reshape(n_etiles, 128, 1)).astype(BF16)
    tt = np.array([[T]], np.float32)

    in_maps = []
    for c in range(N_CORES):
        sl = slice(c * NS, (c + 1) * NS)
        in_maps.append({
            "xh": np.ascontiguousarray(xh[:, sl]),
            "xl": np.ascontiguousarray(xl[:, sl]),
            "wh": wh, "wl": wl, "ws": ws, "bv": bv, "tv": tv, "tt": tt,
        })
    return n_etiles, in_maps


def kernel(x, W, b, alphas, _trace=False, _trace_kwargs=None):
    n_etiles, in_maps = _prep_inputs(
        np.asarray(x), np.asarray(W), np.asarray(b), np.asarray(alphas)
    )
    nc = _program_cache.get(n_etiles)
    if nc is None:
        nc = _build(n_etiles)
        _program_cache[n_etiles] = nc

    kwargs = {}
    if _trace:
        kwargs["trace"] = True
        kwargs.update(_trace_kwargs or {})
    res = run_bass_kernel_spmd(nc, in_maps, core_ids=list(range(N_CORES)), **kwargs)
    out = np.concatenate([res.results[c]["out"] for c in range(N_CORES)])
    if _trace:
        kernel.last_results = res
    return out.astype(np.float32)
